# revision 29
# baseline (speedup 1.0000x reference)
"""Trainium2 Bass kernel for nn_AttentionModule (gnn_message_passing), v4.

Takes FULL inputs, shards batch dim across 8 NeuronCores (pure data
parallel), runs a hand-written Bass/Tile kernel per core, gathers the
full output.

v4 design (vs v2 baseline, 84.6us):
  - x loaded as bf16 via SWDGE cast-DMA (same HBM bytes, half SBUF
    traffic); all on-chip tensors bf16 except PSUM accumulation.
  - transposes run on bf16 (1.0 cyc/row vs 1.5 for fp32r).
  - gating entirely on DVE in bf16 (2x mode); output tile bf16, stored
    via SWDGE cast-DMA bf16 -> fp32.
  - all half-loads issued upfront so the DMA engines stream loads
    back-to-back.
  - variable-width batch chunks (default [t0],[t1],[t2,t3]): narrow
    early chunks get their stores into the DMA stream right after the
    loads; the wide last chunk keeps per-op overheads low.

Self-contained: all shapes/constants hardcoded.
"""

import numpy as np
import ml_dtypes

import concourse.bass as bass
import concourse.bacc as bacc_mod
import concourse.tile as tile
from concourse import mybir
from concourse.bass_utils import run_bass_kernel_spmd

# ---------------- problem constants (hardcoded) ----------------
B, K, C, CI = 4096, 17, 256, 64
NCORES = 8
BC = B // NCORES            # 512 batches per core
R = BC * K                  # 8704 rows per core
NT = BC // 128              # 4 partition-blocks of 128 batches
GROUPS = [[0, 1, 2, 3, 4], [5, 7, 9], [6, 8, 10], [11, 13, 15], [12, 14, 16]]
G = len(GROUPS)
KORDER = [k for g in GROUPS for k in g]          # slot -> original keypoint
SOFF = [0, 5, 8, 11, 14, 17]                     # group offsets in slot space
BN_EPS = 1e-5

# tunables (sim-swept)
CHUNKS = [[0], [1], [2], [3]]   # t-blocks per chunk
SCHEDULE = [("f", 0), ("f", 1), ("e", 0), ("t", 0, 0), ("f", 2), ("e", 1),
            ("t", 1, 0), ("f", 3), ("e", 2), ("t", 2, 0), ("e", 3), ("t", 3, 0)]
PST_BUFS = 2
PXD_BUFS = 3
PPE_BUFS = 3
ACT_COPY_MOD = 3              # legacy knob (unused when COPY_PATTERN set)
COPY_PATTERN = "vvs"          # per-copy engine cycle: v=DVE, s=Act, p=Pool
SCATTER_ENG = "v"             # v=DVE, p=Pool
GATE_ENGS = "vvvvv"           # per-group gating engine
STORE_PIECES = 1              # 1 = full-t store, 2/3 = split
XIN_BUFS = 5                  # >4 lets next iteration's loads start early
LOAD_HALVES = False           # False = one full-t load DMA
OUTP_BUFS = 4

F32 = mybir.dt.float32
BF16 = mybir.dt.bfloat16
AFT = mybir.ActivationFunctionType
ALU = mybir.AluOpType


CWN = 1536  # packed bf16 consts: ident|w1|wpe_dup|wag_dup|wagb|ones


def build_nc(repeat=1):
    nc = bacc_mod.Bacc()
    x_h = nc.declare_dram_parameter("x", [R, C], F32, isOutput=False)
    cw_h = nc.declare_dram_parameter("cw", [128, CWN], BF16, isOutput=False)
    cb_h = nc.declare_dram_parameter("cb", [128, 2], F32, isOutput=False)
    out_h = nc.declare_dram_parameter("out", [R, C], F32, isOutput=True)

    # row r of x = b*K + k with b = t*128 + p  ->  view [t, p, k, c]
    x_r = x_h[:].rearrange("(t p k) c -> t p k c", t=NT, p=128, k=K)
    out_r = out_h[:].rearrange("(t p k) c -> t p k c", t=NT, p=128, k=K)

    # edge lists per target: agg[i] = sum over j != i of
    # silu2(Wp_i^T s_i + W2_j^T s_j + b2)
    JS = [[j for j in range(G) if j != i] for i in range(G)]

    import contextlib
    with tile.TileContext(nc) as tc:
        rep_ctx = (
            tc.For_i(0, repeat, 1, hint_engines=(mybir.EngineType.PE,),
                     staggered_reset=True)
            if repeat > 1 else contextlib.nullcontext()
        )
        with (
            tc.tile_pool(name="consts", bufs=1) as consts,
            tc.tile_pool(name="xin", bufs=XIN_BUFS) as xin,
            tc.tile_pool(name="outp", bufs=OUTP_BUFS) as outp,
            tc.tile_pool(name="xts", bufs=6) as xtsp,
            tc.tile_pool(name="xds", bufs=3) as xdsp,
            tc.tile_pool(name="sums", bufs=3) as sumsp,
            tc.tile_pool(name="pes", bufs=3) as pesp,
            tc.tile_pool(name="aggs", bufs=3) as aggp,
            tc.tile_pool(name="atts", bufs=3) as attp,
            tc.tile_pool(name="pst", bufs=PST_BUFS, space="PSUM") as pstp,
            tc.tile_pool(name="pxd", bufs=PXD_BUFS, space="PSUM") as pxdp,
            tc.tile_pool(name="ppe", bufs=PPE_BUFS, space="PSUM") as ppep,
        ):
            # ---- constants: ONE packed bf16 DMA + one fp32 DMA on the
            # sync (SP) ring, so they land before the x loads and keep the
            # Act sequencer free
            cw_sb = consts.tile([128, CWN], BF16)
            nc.sync.dma_start(out=cw_sb, in_=cw_h[:])
            cb_sb = consts.tile([128, 2], F32)
            nc.sync.dma_start(out=cb_sb, in_=cb_h[:])
            ident_r = cw_sb[:, 0:128]
            w1_sb = cw_sb[:, 128:256]
            wpe_sb = cw_sb[:, 256:896]       # [128, 10*CI], dup rows
            wag_sb = cw_sb[:, 896:1152]      # [128, C], dup rows
            wagb_sb = cw_sb[:, 1152:1408]    # rows 0/64 = b_agg/2
            ones_sb = cw_sb[:, 1408:1536]    # all ones
            b1_sb = cb_sb[:, 0:1]
            b2_sb = cb_sb[:, 1:2]

            # warm the Silu/Tanh activation-table set during the load phase
            # (input = the bias consts, which land within ~1us; no memset -
            # memset lowers to the Pool engine and would delay load desc-gen)
            warm = consts.tile([128, 2], F32)
            nc.scalar.activation(out=warm[:], in_=cb_sb[:], func=AFT.Silu)

            with rep_ctx:
                # ---- all x loads upfront (SWDGE cast fp32 -> bf16) ----
                # per t-block, two half-loads (kp 0:9 and 9:17), in chunk
                # order so early chunks complete first
                xt = {}
                for t in range(NT):
                    xt[t] = xin.tile([128, K * C], BF16, name="xtile")
                load_splits = ((0, 9), (9, K)) if LOAD_HALVES else ((0, K),)
                for ch_ts in CHUNKS:
                    for lo, hi in load_splits:
                        for t in ch_ts:
                            nc.gpsimd.dma_start(
                                out=xt[t].rearrange("p (k c) -> p k c", c=C)[:, lo:hi, :],
                                in_=x_r[t][:, lo:hi, :],
                            )

                ncopy = [0]  # distributes PSUM->SBUF copies between DVE and Act

                # pairs needing only kp<9 first (their half-a loads land first)
                PAIR_ORDER = [0, 1, 2, 4, 3, 5, 6, 7, 8]

                state = {}

                def copy_engine(dst, src):
                    c = COPY_PATTERN[ncopy[0] % len(COPY_PATTERN)]
                    if c == "s":
                        nc.scalar.copy(out=dst, in_=src)
                    elif c == "p":
                        nc.gpsimd.tensor_copy(out=dst, in_=src)
                    else:
                        nc.vector.tensor_copy(out=dst, in_=src)
                    ncopy[0] += 1

                def front(ich):
                    """transpose + down-proj + silu1 + group sums for one chunk.

                    slot s -> original k = KORDER[s]; pair p9 = slots (2p9, 2p9+1)
                    chunk width W = 128 * len(ts); xd_sb column block of slot s
                    = (s//2)*2W, row half 64*(s%2)
                    """
                    ts = CHUNKS[ich]
                    W = 128 * len(ts)
                    xd_sb = xdsp.tile([128, 9 * W], BF16)
                    sums_sb = sumsp.tile([128, G * W], BF16)
                    state[ich] = {"xd": xd_sb, "sums": sums_sb, "W": W}
                    xdq_of = {}
                    pairs_done = set()

                    def xd_ap(s):
                        return xd_sb[64 * (s % 2):64 * (s % 2) + 64,
                                     (s // 2) * W:(s // 2) * W + W]

                    for p9 in PAIR_ORDER:
                        q = p9 // 2
                        if q not in xdq_of:
                            xdq_of[q] = pxdp.tile([128, 2 * W], F32, name="xdq")
                        xdq = xdq_of[q] if q < 4 else xdq_of[q][0:64, 0:W]
                        lp = p9 % 2
                        slots = [2 * p9, 2 * p9 + 1] if p9 < 8 else [16]
                        ns = len(slots)
                        # early pairs (waiting on the second half-load) transpose
                        # per t-block; late pairs have all data - one big copy
                        early = p9 in (0, 1, 2) and len(ts) > 1
                        if len(ts) == 1:
                            # single-t chunk: both ch halves in one pst + copy
                            t = ts[0]
                            pst = pstp.tile([128, 256 * ns], BF16, name="pst")
                            for si, s in enumerate(slots):
                                k = KORDER[s]
                                for ch in range(2):
                                    nc.tensor.transpose(
                                        out=pst[:, (si * 2 + ch) * 128:
                                                (si * 2 + ch + 1) * 128],
                                        in_=xt[t][:, k * C + ch * 128: k * C + ch * 128 + 128],
                                        identity=ident_r,
                                    )
                            xts2 = xtsp.tile([128, 256 * ns], BF16, name="xts")
                            copy_engine(xts2, pst)

                            def rhs_ap(si, ch):
                                return xts2[:, (si * 2 + ch) * 128:
                                            (si * 2 + ch + 1) * 128]
                        else:
                            xts = []
                            for ch in range(2):
                                xts_t = xtsp.tile([128, W * ns], BF16, name="xts")
                                xts_v = xts_t.rearrange("p (s u) -> p s u", s=ns)
                                if early:
                                    for ti, t in enumerate(ts):
                                        pst = pstp.tile([128, 128 * ns], BF16, name="pst")
                                        for si, s in enumerate(slots):
                                            k = KORDER[s]
                                            nc.tensor.transpose(
                                                out=pst[:, si * 128:(si + 1) * 128],
                                                in_=xt[t][:, k * C + ch * 128: k * C + ch * 128 + 128],
                                                identity=ident_r,
                                            )
                                        dst = xts_v[:, :, ti * 128:(ti + 1) * 128]
                                        src = pst.rearrange("p (s u) -> p s u", s=ns)
                                        copy_engine(dst, src)
                                else:
                                    pst = pstp.tile([128, W * ns], BF16, name="pst")
                                    for si, s in enumerate(slots):
                                        k = KORDER[s]
                                        for ti, t in enumerate(ts):
                                            nc.tensor.transpose(
                                                out=pst[:, (si * len(ts) + ti) * 128:
                                                        (si * len(ts) + ti + 1) * 128],
                                                in_=xt[t][:, k * C + ch * 128: k * C + ch * 128 + 128],
                                                identity=ident_r,
                                            )
                                    copy_engine(xts_t, pst)
                                xts.append(xts_t)

                            def rhs_ap(si, ch):
                                return xts[ch][:, si * W:si * W + W]
                        for si, s in enumerate(slots):
                            for ch in range(2):
                                nc.tensor.matmul(
                                    out=xdq[64 * (s % 2):64 * (s % 2) + 64,
                                            lp * W:lp * W + W],
                                    lhsT=w1_sb[:, ch * CI:(ch + 1) * CI],
                                    rhs=rhs_ap(si, ch),
                                    start=(ch == 0), stop=(ch == 1),
                                    skip_group_check=True,
                                )
                        # silu1 fires once both pairs of the bank are done
                        pairs_done.add(p9)
                        bank_pairs = [2 * q, 2 * q + 1] if q < 4 else [8]
                        if all(p in pairs_done for p in bank_pairs):
                            if q < 4:
                                nc.scalar.activation(
                                    out=xd_sb[:, q * 2 * W:(q + 1) * 2 * W],
                                    in_=xdq_of[q],
                                    func=AFT.Silu, bias=b1_sb,
                                )
                            else:
                                nc.scalar.activation(
                                    out=xd_sb[0:64, 8 * W:8 * W + W],
                                    in_=xdq,
                                    func=AFT.Silu, bias=b1_sb[0:64],
                                )

                    # group sums at chunk width, kept as stacked even/odd-slot
                    # partials (rows 0:64 / 64:128); partition-aligned adds only.
                    for g in range(G):
                        slots = list(range(SOFF[g], SOFF[g + 1]))
                        for half in range(2):
                            hs = [s for s in slots if s % 2 == half]
                            sl = sums_sb[64 * half:64 * half + 64,
                                         g * W:(g + 1) * W]
                            if len(hs) == 1:
                                nc.vector.tensor_copy(out=sl, in_=xd_ap(hs[0]))
                            else:
                                nc.vector.tensor_add(out=sl, in0=xd_ap(hs[0]),
                                                     in1=xd_ap(hs[1]))
                                for s in hs[2:]:
                                    nc.vector.tensor_add(out=sl, in0=sl,
                                                         in1=xd_ap(s))

                def edges(ich):
                    """edge conv + scatter-add at chunk width."""
                    sums_sb = state[ich]["sums"]
                    W = state[ich]["W"]
                    nth = W // 128
                    pe_sb = pesp.tile([128, 6 * 2 * W], BF16, name="pe_sb")
                    agg_sb = aggp.tile([128, 3 * W], BF16, name="agg_sb")
                    att_sb = attp.tile([128, nth * G * C], BF16, name="att_sb")
                    state[ich].update(pe=pe_sb, agg=agg_sb, att=att_sb)

                    def pe_col(tgt, e):
                        return ((tgt // 2) * 2 + e // 2) * 2 * W + (e % 2) * W

                    ngrp = 1 if W == 128 else 2   # edge-groups per psum tile
                    epw = 4 // ngrp                # edges per psum tile
                    for tp, tgts in enumerate([(0, 1), (2, 3), (4,)]):
                        rows = 64 * len(tgts)
                        for ep in range(ngrp):
                            pep = ppep.tile([rows, epw * W], F32, name="pep")
                            for rh, tgt in zip((0, 64), tgts):
                                for el in range(epw):
                                    e = ep * epw + el
                                    j = JS[tgt][e]
                                    outap = pep[rh:rh + 64, el * W:el * W + W]
                                    nc.tensor.matmul(
                                        out=outap,
                                        lhsT=wpe_sb[:, tgt * CI:(tgt + 1) * CI],
                                        rhs=sums_sb[:, tgt * W:(tgt + 1) * W],
                                        start=True, stop=False,
                                        skip_group_check=True,
                                    )
                                    nc.tensor.matmul(
                                        out=outap,
                                        lhsT=wpe_sb[:, (G + j) * CI:(G + j + 1) * CI],
                                        rhs=sums_sb[:, j * W:(j + 1) * W],
                                        start=False, stop=True,
                                        skip_group_check=True,
                                    )
                            nc.scalar.activation(
                                out=pe_sb[0:rows, (tp * 4 + ep * epw) * W:
                                          (tp * 4 + (ep + 1) * epw) * W],
                                in_=pep, func=AFT.Silu, bias=b2_sb[0:rows],
                            )

                    # scatter-add, target pairs stacked
                    sceng = nc.vector if SCATTER_ENG == "v" else nc.gpsimd
                    for tp, tgts in enumerate([(0, 1), (2, 3), (4,)]):
                        rows = 64 * len(tgts)
                        sl = agg_sb[0:rows, tp * W:(tp + 1) * W]
                        sceng.tensor_add(
                            out=sl,
                            in0=pe_sb[0:rows, pe_col(tgts[0], 0):pe_col(tgts[0], 0) + W],
                            in1=pe_sb[0:rows, pe_col(tgts[0], 1):pe_col(tgts[0], 1) + W])
                        sceng.tensor_add(
                            out=sl, in0=sl,
                            in1=pe_sb[0:rows, pe_col(tgts[0], 2):pe_col(tgts[0], 2) + W])
                        sceng.tensor_add(
                            out=sl, in0=sl,
                            in1=pe_sb[0:rows, pe_col(tgts[0], 3):pe_col(tgts[0], 3) + W])

                def tail(ich, th):
                    """att + gating + store for one t-block of the chunk."""
                    t = CHUNKS[ich][th]
                    W = state[ich]["W"]
                    agg_sb = state[ich]["agg"]
                    att_sb = state[ich]["att"]

                    xv = xt[t].rearrange("p (k c) -> p k c", c=C)
                    ot = outp.tile([128, K * C], BF16, name="otile")
                    ov = ot.rearrange("p (k c) -> p k c", c=C)

                    def att_mm(gp):
                        gl = 256 * len(gp)
                        patt = ppep.tile([128, 512], F32, name="pep")
                        patt = patt[:, 0:gl]
                        for gi, g in enumerate(gp):
                            rh = 64 * (g % 2)
                            outap = patt[:, gi * 256:gi * 256 + 256]
                            nc.tensor.matmul(
                                out=outap,
                                lhsT=agg_sb[rh:rh + 64,
                                            (g // 2) * W + th * 128:
                                            (g // 2) * W + th * 128 + 128],
                                rhs=wag_sb[rh:rh + 64, :],
                                start=True, stop=False,
                                skip_group_check=True,
                            )
                            nc.tensor.matmul(
                                out=outap,
                                lhsT=ones_sb[rh:rh + 1, :],
                                rhs=wagb_sb[rh:rh + 1, :],
                                start=False, stop=True,
                                skip_group_check=True,
                            )
                        nc.scalar.activation(
                            out=att_sb[:, th * G * C + gp[0] * 256:
                                       th * G * C + gp[0] * 256 + gl],
                            in_=patt, func=AFT.Tanh,
                        )
                        # (tanh+1)*0.5 in place, this slab only
                        sl = att_sb[:, th * G * C + gp[0] * 256:
                                    th * G * C + gp[0] * 256 + gl]
                        nc.vector.tensor_scalar(
                            out=sl, in0=sl, scalar1=1.0, scalar2=0.5,
                            op0=ALU.add, op1=ALU.mult,
                        )

                    def gate(g):
                        ks = GROUPS[g]
                        step = ks[1] - ks[0] if len(ks) > 1 else 1
                        xg = xv[:, ks[0]:ks[-1] + 1:step, :]
                        og = ov[:, ks[0]:ks[-1] + 1:step, :]
                        av = att_sb[:, th * G * C + g * 256:th * G * C + (g + 1) * 256]
                        av = av.rearrange("p (u c) -> p u c", u=1).broadcast_to(
                            (128, len(ks), C))
                        geng = nc.vector if GATE_ENGS[g] == "v" else nc.gpsimd
                        geng.tensor_mul(out=og, in0=xg, in1=av)

                    # per-gp chains so gating starts before the last tanh;
                    # stores fire as soon as the kp range they cover is gated
                    if STORE_PIECES == 3:
                        att_mm((0,))
                        gate(0)
                        nc.gpsimd.dma_start(out=out_r[t][:, 0:5, :],
                                            in_=ov[:, 0:5, :])
                        att_mm((1, 2))
                        gate(1)
                        gate(2)
                        nc.gpsimd.dma_start(out=out_r[t][:, 5:11, :],
                                            in_=ov[:, 5:11, :])
                        att_mm((3, 4))
                        gate(3)
                        gate(4)
                        nc.gpsimd.dma_start(out=out_r[t][:, 11:K, :],
                                            in_=ov[:, 11:K, :])
                    elif STORE_PIECES == 2:
                        att_mm((0, 1))
                        gate(0)
                        gate(1)
                        att_mm((2, 3))
                        gate(2)
                        nc.gpsimd.dma_start(out=out_r[t][:, 0:9, :],
                                            in_=ov[:, 0:9, :])
                        gate(3)
                        att_mm((4,))
                        gate(4)
                        nc.gpsimd.dma_start(out=out_r[t][:, 9:K, :],
                                            in_=ov[:, 9:K, :])
                    else:
                        att_mm((0, 1))
                        gate(0)
                        gate(1)
                        att_mm((2, 3))
                        gate(2)
                        gate(3)
                        att_mm((4,))
                        gate(4)
                        nc.gpsimd.dma_start(out=out_r[t][:, 0:K, :],
                                            in_=ov[:, 0:K, :])

                # pipelined emission (engine streams are in-order, so
                # emission order is stream order: fronts of later chunks
                # must be emitted before earlier chunks' backs)
                for step in SCHEDULE:
                    if step[0] == "f":
                        front(step[1])
                    elif step[0] == "e":
                        edges(step[1])
                    else:
                        tail(step[1], step[2])

    nc.compile()
    return nc


def _prep_weights(W_down, b_down, bn1_scale, bn1_bias, bn1_mean, bn1_var,
                  W_conv, bn2_scale, bn2_bias, bn2_mean, bn2_var, W_agg, b_agg):
    f64 = np.float64
    bf16 = ml_dtypes.bfloat16
    a1 = bn1_scale.astype(f64) / np.sqrt(bn1_var.astype(f64) + BN_EPS)
    W1f = W_down.astype(f64) * a1[None, :]                      # [256, 64]
    b1f = (b_down.astype(f64) - bn1_mean) * a1 + bn1_bias       # [64]

    a2 = bn2_scale.astype(f64) / np.sqrt(bn2_var.astype(f64) + BN_EPS)
    Wc = W_conv.astype(f64) * a2[:, None]                       # [64, 128]
    b2f = bn2_bias.astype(f64) - bn2_mean * a2                  # [64]
    W1, W2 = Wc[:, :CI], Wc[:, CI:]
    Wp = W1 - W2

    GS = [len(g) for g in GROUPS]
    # w1 sbuf layout: [128, 2*CI], col-block ch = W1f[ch*128:(ch+1)*128, :]
    w1 = np.concatenate([W1f[:128, :], W1f[128:, :]], axis=1)

    # wpe: [64, 10*64]: blocks 0..4 = Wp.T/|g_i|, 5..9 = W2.T/|g_j|
    blocks = [Wp.T / GS[i] for i in range(G)] + [W2.T / GS[j] for j in range(G)]
    wpe = np.concatenate(blocks, axis=1)                        # [64, 640]
    wpe_dup = np.concatenate([wpe, wpe], axis=0)                # [128, 640]

    # wag carries the 1/2 of sigmoid(z) = 0.5*tanh(z/2)+0.5
    wag = 0.5 * W_agg.astype(f64)                               # [64, 256]
    wag_dup = np.concatenate([wag, wag], axis=0)                # [128, 256]
    wagb = np.broadcast_to(0.5 * b_agg.astype(f64)[None, :], (128, C))

    # packed bf16 const block: ident|w1|wpe_dup|wag_dup|wagb|ones
    cw = np.concatenate([
        np.eye(128), w1, wpe_dup, wag_dup, wagb, np.ones((128, 128)),
    ], axis=1).astype(bf16)                                     # [128, 1536]
    cb = np.concatenate([
        np.tile(b1f.reshape(CI, 1), (2, 1)),
        np.tile(b2f.reshape(CI, 1), (2, 1)),
    ], axis=1).astype(np.float32)                               # [128, 2]
    return cw, cb


_NC_CACHE = {}


def _run(inputs, trace=False, trace_kwargs=None):
    x = np.ascontiguousarray(np.asarray(inputs["x_bk_c"], dtype=np.float32))
    assert x.shape == (B * K, C), x.shape
    cw, cb = _prep_weights(
        np.asarray(inputs["W_down"]), np.asarray(inputs["b_down"]),
        np.asarray(inputs["bn1_scale"]), np.asarray(inputs["bn1_bias"]),
        np.asarray(inputs["bn1_mean"]), np.asarray(inputs["bn1_var"]),
        np.asarray(inputs["W_conv"]),
        np.asarray(inputs["bn2_scale"]), np.asarray(inputs["bn2_bias"]),
        np.asarray(inputs["bn2_mean"]), np.asarray(inputs["bn2_var"]),
        np.asarray(inputs["W_agg"]), np.asarray(inputs["b_agg"]),
    )

    if "nc" not in _NC_CACHE:
        _NC_CACHE["nc"] = build_nc()
    nc = _NC_CACHE["nc"]

    in_maps = []
    for c in range(NCORES):
        in_maps.append({
            "x": np.ascontiguousarray(x[c * R:(c + 1) * R]),
            "cw": cw, "cb": cb,
        })
    kw = {}
    if trace:
        kw["trace"] = True
        if trace_kwargs:
            kw["trace_kwargs"] = trace_kwargs
    res = run_bass_kernel_spmd(nc, in_maps, core_ids=list(range(NCORES)), **kw)
    out = np.concatenate([r["out"] for r in res.results], axis=0)
    return out, res


def kernel(**inputs) -> np.ndarray:
    out, _ = _run(inputs)
    return out
